# revision 45
# baseline (speedup 1.0000x reference)
"""Trainium2 Bass kernel for nn_LocalModel (6-encoder local-attention transformer).

Sharding: data-parallel over batch — B=8 batch elements, one per NeuronCore.
Each core runs the full 6-layer encoder stack + final projection for its
batch element entirely on-chip (all weights resident in SBUF), returning a
[6]-vector; the host gathers them into the [8, 6] output.

Attention uses the zero-masked-softmax identity: with out-of-window scores
set to 0 (not -inf), softmax over the full sequence satisfies
    out_i = (sum_{j in W} (e^{s_ij} - 1) v_j + sum_all v_j)
          / (sum_{j in W} (e^{s_ij} - 1) + S)
so only the 128-wide banded scores are ever computed. The "+sum_all v / +S"
terms enter the PSUM accumulation via a K=1 matmul against an augmented
V-total row (64 v-columns + a ones-column per head).
"""
import sys
import numpy as np

sys.path.insert(0, "/opt/trn_rl_repo")

B, S, D = 8, 1024, 512
H, Dh, W = 8, 64, 64
HD = 2048           # ffn hidden
C = 6               # classes
ENC = 6
EPS = 1e-5
P = 128
KO = D // P         # 4
HC = HD // P        # 16
SCALE = Dh ** -0.5

_CACHE = {}
LAST_EXEC_NS = None
LAST_RESULTS = None
TRACE = False


def _build(affine: bool):
    import concourse.bass as bass
    import concourse.tile as tile
    from concourse import bacc, mybir
    from concourse.masks import make_identity

    f32 = mybir.dt.float32
    f32r = mybir.dt.float32r
    bf16 = mybir.dt.bfloat16
    AF = mybir.ActivationFunctionType
    OP = mybir.AluOpType

    nc = bacc.Bacc()
    d = {}
    d['xT'] = nc.declare_dram_parameter("xT", [D, S], bf16, isOutput=False)
    for w in ("wqT", "wkT", "wvT"):
        d[w] = nc.declare_dram_parameter(w, [D, D], bf16, isOutput=False)
    for b_ in ("bq", "bk"):
        d[b_] = nc.declare_dram_parameter(b_, [D], f32, isOutput=False)
    d['bv'] = nc.declare_dram_parameter("bv", [D], f32, isOutput=False)
    d['fc1T'] = nc.declare_dram_parameter("fc1T", [D, HD], bf16, isOutput=False)
    d['fc1b'] = nc.declare_dram_parameter("fc1b", [HD], f32, isOutput=False)
    d['fc2T'] = nc.declare_dram_parameter("fc2T", [HD, D], bf16, isOutput=False)
    d['fc2b'] = nc.declare_dram_parameter("fc2b", [D], f32, isOutput=False)
    d['mask'] = nc.declare_dram_parameter("mask", [P, 384], f32, isOutput=False)
    d['ow'] = nc.declare_dram_parameter("ow", [C, S, D], f32, isOutput=False)
    if affine:
        d['lng'] = nc.declare_dram_parameter("lng", [D], f32, isOutput=False)
        d['lnb'] = nc.declare_dram_parameter("lnb", [D], f32, isOutput=False)
    out_d = nc.declare_dram_parameter("out", [1, C], f32, isOutput=True)

    def bcast_ap(dram_h, parts=P):
        # replicate a [N] dram vector across `parts` partitions
        a = dram_h[:]
        return bass.AP(tensor=a.tensor, offset=a.offset,
                       ap=[[0, parts]] + [list(x) for x in a.ap])

    from contextlib import ExitStack
    with tile.TileContext(nc) as tc, ExitStack() as ctx:
        wpool = ctx.enter_context(tc.tile_pool(name="wpool", bufs=1))
        big = ctx.enter_context(tc.tile_pool(name="big", bufs=2))
        qkp = ctx.enter_context(tc.tile_pool(name="qkp", bufs=1))
        vap = ctx.enter_context(tc.tile_pool(name="vap", bufs=1))
        atp = ctx.enter_context(tc.tile_pool(name="atp", bufs=1))
        hp = ctx.enter_context(tc.tile_pool(name="hp", bufs=1))
        pp = ctx.enter_context(tc.tile_pool(name="pp", bufs=4))
        tmp = ctx.enter_context(tc.tile_pool(name="tmp", bufs=6))
        small = ctx.enter_context(tc.tile_pool(name="small", bufs=4))
        x1p = ctx.enter_context(tc.tile_pool(name="x1p", bufs=1))
        owp = ctx.enter_context(tc.tile_pool(name="owp", bufs=4))
        scrp = ctx.enter_context(tc.tile_pool(name="scrp", bufs=2))
        vcp = ctx.enter_context(tc.tile_pool(name="vcp", bufs=1))
        psA = ctx.enter_context(tc.tile_pool(name="psA", bufs=3, space="PSUM"))
        psS = ctx.enter_context(tc.tile_pool(name="psS", bufs=3, space="PSUM"))
        psV = ctx.enter_context(tc.tile_pool(name="psV", bufs=2, space="PSUM"))

        # ---- persistent loads ----
        # xT + wv first (first matmuls need them); fc1/fc2 last so they
        # overlap with early compute. Spread across engine DMA queues.
        xT = big.tile([P, KO, S], bf16, tag="big")
        nc.sync.dma_start(xT, d['xT'].rearrange("(ko p) n -> p ko n", p=P))
        wq_sb = wpool.tile([P, KO, D], bf16, tag="wq")
        wk_sb = wpool.tile([P, KO, D], bf16, tag="wk")
        wv_sb = wpool.tile([P, KO, D], bf16, tag="wv")
        nc.scalar.dma_start(wv_sb, d['wvT'].rearrange("(ko p) n -> p ko n", p=P))
        nc.scalar.dma_start(wq_sb, d['wqT'].rearrange("(ko p) n -> p ko n", p=P))
        nc.scalar.dma_start(wk_sb, d['wkT'].rearrange("(ko p) n -> p ko n", p=P))
        bq_sb = wpool.tile([P, KO], f32, tag="bq")
        bk_sb = wpool.tile([P, KO], f32, tag="bk")
        nc.scalar.dma_start(bq_sb, d['bq'].rearrange("(ko p) -> p ko", p=P))
        nc.scalar.dma_start(bk_sb, d['bk'].rearrange("(ko p) -> p ko", p=P))
        bv_bc = wpool.tile([P, D], f32, tag="bv")
        nc.gpsimd.dma_start(out=bv_bc, in_=bcast_ap(d['bv']))
        mask_sb = wpool.tile([P, 384], f32, tag="mask")
        nc.scalar.dma_start(mask_sb, d['mask'][:])
        fc1_sb = wpool.tile([P, KO, HD], bf16, tag="fc1")
        nc.sync.dma_start(fc1_sb, d['fc1T'].rearrange("(ko p) n -> p ko n", p=P))
        fc2_sb = wpool.tile([P, HC, D], bf16, tag="fc2")
        nc.sync.dma_start(fc2_sb, d['fc2T'].rearrange("(hc p) n -> p hc n", p=P))
        fc1b_sb = wpool.tile([P, HC], f32, tag="fc1b")
        nc.scalar.dma_start(fc1b_sb, d['fc1b'].rearrange("(hc p) -> p hc", p=P))
        fc2b_bc = wpool.tile([P, D], f32, tag="fc2b")
        nc.gpsimd.dma_start(out=fc2b_bc, in_=bcast_ap(d['fc2b']))
        if affine:
            g_bc = wpool.tile([P, D], f32, tag="g")
            b_bc = wpool.tile([P, D], f32, tag="b")
            nc.gpsimd.dma_start(out=g_bc, in_=bcast_ap(d['lng']))
            nc.gpsimd.dma_start(out=b_bc, in_=bcast_ap(d['lnb']))
        ident = wpool.tile([P, P], f32, tag="id")
        make_identity(nc, ident)
        ones_col = wpool.tile([P, 1], f32, tag="onc")
        nc.vector.memset(ones_col, 1.0)
        ones_colb = wpool.tile([P, 1], bf16, tag="oncb")
        nc.vector.memset(ones_colb, 1.0)
        ones_rowb = wpool.tile([1, P], bf16, tag="onrb")
        nc.vector.memset(ones_rowb, 1.0)
        eps_sb = wpool.tile([P, 1], f32, tag="eps")
        nc.vector.memset(eps_sb, EPS)
        vtot_bf = wpool.tile([1, H * 65], bf16, tag="vtot")
        bv1k = wpool.tile([1, D], f32, tag="bv1k")
        nc.scalar.mul(out=bv1k, in_=bv_bc[0:1, :], mul=float(S))
        acc_fp = wpool.tile([P, 8 * C], f32, tag="accfp")
        racc = wpool.tile([P, C], f32, tag="racc")
        nc.vector.memset(racc, 0.0)

        def layer_norm_to(src_ap, out_tile):
            """LayerNorm src [P,512] -> out_tile [P,512] (token-major)."""
            st = small.tile([P, 6], f32, tag="st")
            mv = small.tile([P, 2], f32, tag="mv")
            nc.vector.bn_stats(out=st, in_=src_ap)
            nc.vector.bn_aggr(out=mv, in_=st)
            rstd = small.tile([P, 1], f32, tag="rs")
            nc.scalar.activation(out=rstd, in_=mv[:, 1:2], func=AF.Sqrt,
                                 bias=eps_sb[:, 0:1])
            nc.vector.reciprocal(out=rstd, in_=rstd)
            nc.vector.tensor_scalar(out=out_tile, in0=src_ap,
                                    scalar1=mv[:, 0:1], scalar2=rstd,
                                    op0=OP.subtract, op1=OP.mult)
            if affine:
                nc.vector.tensor_tensor(out=out_tile, in0=out_tile, in1=g_bc,
                                        op=OP.mult)
                nc.vector.tensor_tensor(out=out_tile, in0=out_tile, in1=b_bc,
                                        op=OP.add)

        def transpose_to(src_tile, dst_tile, tb, dve=False):
            """src [P, 512] token-major block tb -> dst [P, KO, S] feature-major."""
            for dc in range(KO):
                pt = psA.tile([P, 512], f32, tag="pj")
                nc.tensor.transpose(pt[:, :P],
                                    src_tile[:, dc * P:(dc + 1) * P], ident)
                dst = dst_tile[:, dc, tb * P:(tb + 1) * P]
                if dve:
                    nc.vector.tensor_scalar_add(out=dst, in0=pt[:, :P],
                                                scalar1=0.0)
                else:
                    nc.scalar.copy(out=dst, in_=pt[:, :P])

        for L in range(ENC):
            # ---------- QKV projections ----------
            va = vap.tile([P, 8, H, 65], bf16, tag="va")
            nc.vector.memset(va[:, :, :, 64:65], 1.0)
            # V first (frees xT earliest), token-major
            for tb in range(8):
                pv = psA.tile([P, 512], f32, tag="pj")
                for ko in range(KO):
                    nc.tensor.matmul(
                        pv, lhsT=xT[:, ko, tb * P:(tb + 1) * P],
                        rhs=wv_sb[:, ko, :],
                        start=(ko == 0), stop=(ko == KO - 1))
                nc.vector.tensor_tensor(
                    out=va[:, tb, :, 0:64],
                    in0=pv.rearrange("p (h a) -> p h a", a=64),
                    in1=bv_bc.rearrange("p (h a) -> p h a", a=64),
                    op=OP.add)
            # q/k per d'-chunk tiles (bf16) so scores can start per head pair
            q_t, k_t = [], []
            for mc in range(KO):
                qm = qkp.tile([P, S], bf16, tag=f"q{mc}")
                km = qkp.tile([P, S], bf16, tag=f"k{mc}")
                q_t.append(qm)
                k_t.append(km)
                for half in range(2):
                    cs = slice(half * 512, (half + 1) * 512)
                    pq = psA.tile([P, 512], f32, tag="pj")
                    for ko in range(KO):
                        nc.tensor.matmul(
                            pq, lhsT=wq_sb[:, ko, mc * P:(mc + 1) * P],
                            rhs=xT[:, ko, cs],
                            start=(ko == 0), stop=(ko == KO - 1))
                    nc.vector.tensor_scalar_add(out=qm[:, cs], in0=pq,
                                                scalar1=bq_sb[:, mc:mc + 1])
                    pk = psA.tile([P, 512], f32, tag="pj")
                    for ko in range(KO):
                        nc.tensor.matmul(
                            pk, lhsT=wk_sb[:, ko, mc * P:(mc + 1) * P],
                            rhs=xT[:, ko, cs],
                            start=(ko == 0), stop=(ko == KO - 1))
                    nc.vector.tensor_scalar_add(out=km[:, cs], in0=pk,
                                                scalar1=bk_sb[:, mc:mc + 1])

            # ---------- V totals: (sum_t x) @ wvT + S*bv ----------
            xs32 = small.tile([P, KO], f32, tag="xs")
            nc.vector.reduce_sum(out=xs32, in_=xT,
                                 axis=mybir.AxisListType.X)
            xsr = small.tile([P, KO], bf16, tag="xsr")
            nc.scalar.copy(out=xsr, in_=xs32)
            pvt = psV.tile([1, D], f32, tag="av")
            for ko in range(KO):
                nc.tensor.matmul(pvt, lhsT=xsr[:, ko:ko + 1],
                                 rhs=wv_sb[:, ko, :],
                                 start=(ko == 0), stop=(ko == KO - 1))
            nc.vector.tensor_tensor(
                out=vtot_bf.rearrange("p (h a) -> p h a", a=65)[:, :, 0:64],
                in0=pvt.rearrange("p (h a) -> p h a", a=64),
                in1=bv1k.rearrange("p (h a) -> p h a", a=64), op=OP.add)
            nc.vector.memset(
                vtot_bf.rearrange("p (h a) -> p h a", a=65)[:, :, 64:65],
                float(S))
            # vcorr[qb] = vtot - sum_{kc in W(qb)} colsum(va[kc])
            # negated chunk colsums csn[kc] (base-0 [1,520] tiles), then
            # 4 accumulate-matmuls per (qb, half) into vcb_q[qb].
            csn = [vcp.tile([1, H * 65], bf16, tag=f"csn{kc}",
                            name=f"csn{kc}") for kc in range(8)]
            vcb_q = [vcp.tile([1, H * 65], bf16, tag=f"vcb{qb}",
                              name=f"vcb{qb}") for qb in range(8)]
            one1 = ones_rowb[0:1, 0:1]

            def vcorr_calc():
                for g in range(2):
                    gs = slice(260 * g, 260 * (g + 1))
                    for kc in range(8):
                        pcsk = psV.tile([1, 260], f32, tag="av")
                        nc.tensor.matmul(pcsk, lhsT=ones_colb[:, 0:1],
                                         rhs=va[:, kc, 4 * g:4 * g + 4, :],
                                         start=True, stop=True)
                        nc.scalar.mul(out=csn[kc][0:1, gs], in_=pcsk,
                                      mul=-1.0)
                for qb in range(8):
                    kcs = [kc for kc in (qb - 1, qb, qb + 1) if 0 <= kc < 8]
                    for g in range(2):
                        gs = slice(260 * g, 260 * (g + 1))
                        pvq = psV.tile([1, 260], f32, tag="av")
                        nc.tensor.matmul(pvq, lhsT=one1,
                                         rhs=vtot_bf[0:1, gs],
                                         start=True, stop=False)
                        for i, kc in enumerate(kcs):
                            nc.tensor.matmul(pvq, lhsT=one1,
                                             rhs=csn[kc][0:1, gs],
                                             start=False,
                                             stop=(i == len(kcs) - 1))
                        nc.scalar.copy(out=vcb_q[qb][0:1, gs], in_=pvq)

            # ---------- attention (kc-major scores, deferred qb consumption) ----
            last = (L == ENC - 1)
            if last:
                # stream the out_w slabs in during the last layer's FFN;
                # consumed (in issue order) by the final dot-products below.
                ow_tiles = []
                for tb in range(8):
                    for r in range(C):
                        owt = owp.tile([P, D], f32, tag="ow")
                        nc.sync.dma_start(
                            owt, d['ow'][r, tb * P:(tb + 1) * P, :])
                        ow_tiles.append(owt)
            a_tok = atp.tile([P, 8, D], f32, tag="at")
            x1T = big.tile([P, KO, S], bf16, tag="big")
            x1toks = [None] * 8
            pc_tiles = {}

            def scores_block(kc):
                q0 = max(0, kc - 1) * P
                q1 = min(8, kc + 2) * P
                qw = q1 - q0
                if kc == 0:
                    mk = mask_sb[:, 128:384]
                elif kc == 7:
                    mk = mask_sb[:, 0:256]
                else:
                    mk = mask_sb[:, :]
                tiles = {}
                for h in range(H):
                    hr = slice(64 * (h % 2), 64 * (h % 2) + 64)
                    hko = h // 2
                    ps = psS.tile([P, 384], f32, tag="s")
                    nc.tensor.matmul(
                        ps[:, :qw],
                        lhsT=k_t[hko][hr, kc * P:(kc + 1) * P],
                        rhs=q_t[hko][hr, q0:q1],
                        start=True, stop=True)
                    # premask: s <- s*mask so exp gives 1 off-band (corrected
                    # by the vcorr rank-1 term in the pav accumulation)
                    nc.vector.tensor_tensor(out=ps[:, :qw], in0=ps[:, :qw],
                                            in1=mk, op=OP.mult)
                    pc = pp.tile([P, 384], bf16, tag=f"p{h}")
                    nc.scalar.activation(out=pc[:, :qw], in_=ps[:, :qw],
                                         func=AF.Exp, scale=SCALE)
                    tiles[h] = pc
                pc_tiles[kc] = tiles

            def consume_qb(qb):
                kcs = [kc for kc in (qb - 1, qb, qb + 1) if 0 <= kc < 8]
                for g in range(2):
                    pav4 = psV.tile([P, 260], f32, tag="av")
                    rc4 = small.tile([P, 4], f32, tag="rc")
                    for hh in range(4):
                        h = 4 * g + hh
                        pav = pav4[:, hh * 65:hh * 65 + 65]
                        for i, kc in enumerate(kcs):
                            off = (qb - max(0, kc - 1)) * P
                            nc.tensor.matmul(
                                pav, lhsT=pc_tiles[kc][h][:, off:off + P],
                                rhs=va[:, kc, h, :],
                                start=(i == 0), stop=False)
                        nc.tensor.matmul(
                            pav, lhsT=ones_rowb[0:1, :],
                            rhs=vcb_q[qb][0:1, h * 65:(h + 1) * 65],
                            start=False, stop=True)
                    for hh in range(4):
                        h = 4 * g + hh
                        pav = pav4[:, hh * 65:hh * 65 + 65]
                        nc.vector.reciprocal(out=rc4[:, hh:hh + 1],
                                             in_=pav[:, 64:65])
                        nc.scalar.mul(out=a_tok[:, qb, h * 64:(h + 1) * 64],
                                      in_=pav[:, 0:64], mul=rc4[:, hh:hh + 1])
                # LN1 for this block; transpose deferred two blocks (the
                # deferral hides the LN chain behind pav matmuls)
                xn = x1p.tile([P, D], f32, tag=f"x1_{qb}")
                layer_norm_to(a_tok[:, qb, :], xn)
                x1toks[qb] = xn
                if qb >= 2:
                    transpose_to(x1toks[qb - 2], x1T, qb - 2)

            for kc in range(8):
                scores_block(kc)
                if kc == 0:
                    # emitted right after the first score block: its small
                    # matmul->copy chains overlap the premask/exp work and
                    # vcb_q[0] is ready before the first pav consumes it
                    vcorr_calc()
                if kc >= 2:
                    consume_qb(kc - 2)
            consume_qb(6)
            consume_qb(7)

            # ---------- FFN + residual + LN2 -> next xT ----------
            if not last:
                xT_next = big.tile([P, KO, S], bf16, tag="big")
            # two attention-phase transposes still pending at FFN entry;
            # they are flushed among the first fc1 blocks (fc1 chunk 0 only
            # reads x1T tokens 0..511, so blocks 6/7 may land late)
            pend = [(x1toks[6], x1T, 6, False), (x1toks[7], x1T, 7, False)]
            for tq2 in range(2):
                qs = slice(tq2 * 512, (tq2 + 1) * 512)
                hts = []
                for hc in range(HC):
                    ph = psA.tile([P, 512], f32, tag="pj")
                    for ko in range(KO):
                        nc.tensor.matmul(
                            ph,
                            lhsT=fc1_sb[:, ko, hc * P:(hc + 1) * P],
                            rhs=x1T[:, ko, qs],
                            start=(ko == 0), stop=(ko == KO - 1))
                    ht = hp.tile([P, 512], bf16, tag=f"h{hc}")
                    nc.scalar.activation(out=ht, in_=ph, func=AF.Relu,
                                         bias=fc1b_sb[:, hc:hc + 1])
                    hts.append(ht)
                    # interleave deferred transposes among the fc1 blocks
                    if pend and hc in (3, 7, 11):
                        src, dst, t2, dve = pend.pop(0)
                        transpose_to(src, dst, t2, dve=dve)
                for tb2 in range(4):
                    tb = tq2 * 4 + tb2
                    pf = psA.tile([P, 512], f32, tag="pj")
                    for hc in range(HC):
                        nc.tensor.matmul(
                            pf, lhsT=hts[hc][:, tb2 * P:(tb2 + 1) * P],
                            rhs=fc2_sb[:, hc, :],
                            start=(hc == 0), stop=(hc == HC - 1))
                    f = tmp.tile([P, D], f32, tag="xn")
                    nc.vector.tensor_tensor(out=f, in0=pf, in1=fc2b_bc, op=OP.add)
                    nc.vector.tensor_tensor(out=f, in0=f, in1=x1toks[tb],
                                            op=OP.add)
                    xn2 = tmp.tile([P, D], f32, tag="xn")
                    layer_norm_to(f, xn2)
                    if last:
                        # final projection partials (fused mult+row-sum):
                        # acc_fp[:, tb*C+r] = sum_d xn2[t, d] * ow[r, tb, t, d]
                        for r in range(C):
                            scr = scrp.tile([P, D], f32, tag="scr")
                            nc.vector.scalar_tensor_tensor(
                                out=scr, in0=xn2, scalar=1.0,
                                in1=ow_tiles[tb * C + r],
                                op0=OP.mult, op1=OP.mult,
                                accum_out=acc_fp[:, tb * C + r:tb * C + r + 1])
                        nc.vector.tensor_tensor(
                            out=racc, in0=racc,
                            in1=acc_fp[:, tb * C:(tb + 1) * C], op=OP.add)
                    else:
                        pend.append((xn2, xT_next, tb, True))
                        if tb2 >= 2:
                            src, dst, t2, dve = pend.pop(0)
                            transpose_to(src, dst, t2, dve=dve)
            if not last:
                for src, dst, t2, dve in pend:
                    transpose_to(src, dst, t2, dve=dve)
                xT = xT_next

        # ---------- final reduce: out[r] = sum_p racc ----------
        pout = psV.tile([1, 260], f32, tag="av")
        nc.tensor.matmul(pout[0:1, 0:C], lhsT=ones_col[:, 0:1], rhs=racc,
                         start=True, stop=True)
        osb = wpool.tile([1, C], f32, tag="osb")
        nc.scalar.copy(out=osb, in_=pout[0:1, 0:C])
        nc.sync.dma_start(out_d[:], osb)

    nc.compile()
    return nc


def _prep(inputs):
    """Host-side input prep shared across cores. Returns (common, per_core, affine)."""
    from ml_dtypes import bfloat16
    emb = np.asarray(inputs['emb'], dtype=np.float32)
    idx = np.asarray(inputs['inputs'])
    pos = np.arange(S, dtype=np.float32)[:, None]
    div = np.exp(-np.log(10000.0) * np.arange(0, D, 2, dtype=np.float32) / D)
    ang = pos * div
    pe = np.zeros((S, D), dtype=np.float32)
    pe[:, 0::2] = np.sin(ang)
    pe[:, 1::2] = np.cos(ang)
    x0 = emb[idx] + pe[None]  # [B, S, D]

    jj = np.arange(P)[:, None]
    ccols = np.arange(384)[None, :]
    delta = 128 + jj - ccols
    mask = ((delta >= -W) & (delta < W)).astype(np.float32)
    mask = np.ascontiguousarray(mask)

    ln_g = np.asarray(inputs['ln_g'], dtype=np.float32)
    ln_b = np.asarray(inputs['ln_b'], dtype=np.float32)
    affine = not (np.all(ln_g == 1.0) and np.all(ln_b == 0.0))

    out_w = np.asarray(inputs['out_w'], dtype=np.float32)
    ow = np.ascontiguousarray(out_w.reshape(C, S, D))  # [C, S, D]

    common = {
        'wqT': np.ascontiguousarray(np.asarray(inputs['wq'], np.float32).T.astype(bfloat16)),
        'wkT': np.ascontiguousarray(np.asarray(inputs['wk'], np.float32).T.astype(bfloat16)),
        'wvT': np.ascontiguousarray(np.asarray(inputs['wv'], np.float32).T.astype(bfloat16)),
        'bq': np.ascontiguousarray(np.asarray(inputs['bq'], np.float32)),
        'bk': np.ascontiguousarray(np.asarray(inputs['bk'], np.float32)),
        'bv': np.ascontiguousarray(np.asarray(inputs['bv'], np.float32)),
        'fc1T': np.ascontiguousarray(np.asarray(inputs['fc1_w'], np.float32).T.astype(bfloat16)),
        'fc1b': np.ascontiguousarray(np.asarray(inputs['fc1_b'], np.float32)),
        'fc2T': np.ascontiguousarray(np.asarray(inputs['fc2_w'], np.float32).T.astype(bfloat16)),
        'fc2b': np.ascontiguousarray(np.asarray(inputs['fc2_b'], np.float32)),
        'mask': mask,
        'ow': ow,
    }
    if affine:
        common['lng'] = np.ascontiguousarray(ln_g)
        common['lnb'] = np.ascontiguousarray(ln_b)
    per_core = [
        {'xT': np.ascontiguousarray(x0[b].T.astype(bfloat16))}
        for b in range(B)
    ]
    return common, per_core, affine


def kernel(**inputs):
    global LAST_EXEC_NS, LAST_RESULTS
    from concourse.bass_utils import run_bass_kernel_spmd

    common, per_core, affine = _prep(inputs)
    if affine not in _CACHE:
        _CACHE[affine] = _build(affine)
    nc = _CACHE[affine]

    in_maps = [dict(common, **pc) for pc in per_core]
    res = run_bass_kernel_spmd(nc, in_maps, list(range(B)), trace=TRACE)
    LAST_EXEC_NS = res.exec_time_ns
    LAST_RESULTS = res
    out = np.stack([res.results[b]["out"][0] for b in range(B)], axis=0)
    out = out + np.asarray(inputs['out_b'], np.float32)[None, :]
    return out.astype(np.float32)

